# revision 1
# baseline (speedup 1.0000x reference)
"""MCR2 variational loss on 8 Trainium2 NeuronCores.

Strategy (data-parallel over the sample axis n):
  - The heavy part of the loss is the per-class second-moment matrices
    M_j = Z^T diag(Pi_j) Z (plus the global gram Z^T Z), which reads all of
    Z/Pi once -> memory-bound. Everything downstream (logdet, log1p terms,
    Frobenius distance) is O(C*d^2) scalar work done on the host in fp64.
  - Fast path (Pi exactly one-hot): each sample contributes to exactly one
    class, so per-class partial grams over class-sorted rows give all M_j,
    and gram = sum_j M_j. Host distributes rows so every core gets an
    almost equal share of each class, pads each class segment to a 128-row
    multiple, and the device accumulates each class's gram in its PSUM
    slice: fp8e4m3 DoubleRow matmuls crunch 256 rows per instruction (two
    128-row subchunks packed per partition), with one plain fp8 matmul for
    a class's odd trailing subchunk. fp8 keeps the final losses within
    ~1.6e-3 relative (measured), far inside the 2e-2 gate, while quartering
    HBM traffic vs fp32 and doubling PE throughput. DMA tiles ramp up in
    size and alternate between the two HWDGE rings (SP + ACT), sized so
    ring delivery (~150GB/s each) stays ahead of the PE (~252GB/s demand).
    Output per core: [128, C*128] fp16 partial M; host all-reduces in fp64.
  - Fallback (general dense Pi): host BLAS contraction.
"""

import numpy as np

EPS = 0.5
MU = 1.0
C = 10
N_TOTAL = 131072
D = 128
N_CORES = 8
CHUNK = 128  # rows per subchunk (PE partition/contraction dim)

_compiled_cache = {}


def _matmul_plan(seg_sub):
    """Per-class unit decomposition: DoubleRow units of 2 subchunks first,
    then a plain single-subchunk matmul when the class length is odd. Each
    class's PSUM accumulation group stays CONTIGUOUS in the instruction
    stream — interleaving groups (e.g. starting all classes up front)
    corrupts earlier partial sums on hardware (measured 9e-2 error).

    Returns (plan, plain_pos, dr_pos): plan is a position-sorted list of
    (global_subchunk_pos, size, class, is_first, is_last); plain_pos maps
    class -> subchunk index of its plain chunk; dr_pos maps class -> first
    subchunk of its DoubleRow block."""
    plain_pos = {}
    dr_pos = {}
    plan = []
    pos = 0
    for j, s in enumerate(seg_sub):
        ndr = s // 2
        dr_pos[j] = pos
        for u in range(ndr):
            plan.append((pos + 2 * u, 2, j, u == 0, s % 2 == 0 and u == ndr - 1))
        if s % 2 == 1:
            plain_pos[j] = pos + 2 * ndr
            plan.append((plain_pos[j], 1, j, ndr == 0, True))
        pos += s
    return plan, plain_pos, dr_pos


def _dma_tile_sizes(seg_sub):
    """Ramped tile sizes (in subchunks), alternating the two HWDGE rings.
    Boundaries never split a DoubleRow unit: with the plain chunks laid out
    first, any boundary inside the plain prefix is fine, and past it any
    even offset is fine (DR blocks have even length). The 130-subchunk
    shape uses a schedule tuned so each ring's next tile lands just before
    the PE needs it."""
    total = sum(seg_sub)
    legal = set()
    pos = 0
    for s in seg_sub:
        for k in range(0, s, 2):
            legal.add(pos + k)
        legal.add(pos + s)
        pos += s
    if total == 130:
        cand = [6, 7, 10, 16, 17, 19, 20, 17, 18]
        if all(sum(cand[: i + 1]) in legal for i in range(len(cand))):
            return cand
    # generic fallback: grow toward 16, snapping to legal boundaries
    sizes = []
    target = [4, 8, 12, 16]
    left = total
    prev = 0
    ti = 0
    while left > 0:
        want = target[min(ti, len(target) - 1)]
        ti += 1
        t = min(want, left)
        # snap prev+t down to a legal boundary
        while t > 1 and (prev + t) not in legal:
            t -= 1
        sizes.append(t)
        prev += t
        left -= t
    return sizes


def _build_bass_program(seg_sub):
    """SPMD bass program computing per-class partial grams.

    seg_sub: list of C ints — 128-row subchunks per class (identical on all
    cores; zero padded on the host). Device input "z" is the class-sorted,
    padded, PRE-TILED Z in fp8e4m3: for each DMA tile t of tsz subchunks, a
    contiguous [128, tsz*128] block (each SBUF partition's data contiguous
    in DRAM). Output "m_out": [128, C*128] fp16 partial M."""
    import concourse.bacc as bacc
    import concourse.tile as tile
    from concourse import mybir
    from contextlib import ExitStack

    total_sub = sum(seg_sub)
    tile_sizes = _dma_tile_sizes(seg_sub)
    plan, _, _ = _matmul_plan(seg_sub)

    # bank groups: classes [0..3] / [4..7] / [8] / [9]; separate PSUM tiles
    # so each group's drain depends only on that group's matmuls, and only a
    # single class's 256B/partition copy+store trails the last matmul
    groups = [(0, 4), (4, 8), (8, 9), (9, C)]

    nc = bacc.Bacc("TRN2", target_bir_lowering=False, debug=False, num_devices=N_CORES)
    z = nc.dram_tensor(
        "z", [total_sub * CHUNK, D], mybir.dt.float8e4, kind="ExternalInput"
    ).ap()
    # fp16 partials: |entry| <= ~2.5k fits easily, the 2^-11 rounding is far
    # below the fp8-input noise floor, and the store bytes halve
    out = nc.dram_tensor(
        "m_out", [D, C * D], mybir.dt.float16, kind="ExternalOutput"
    ).ap()

    # tile 0 loads via a RAW pre-context DMA: the TileContext body only
    # starts after a ~0.9us mini-barrier + branch, but the DMA rings are
    # configured well before that, so issuing the first tile's transfer
    # ahead of the context starts its flight ~1.5us earlier. The PE is
    # gated on the manual completion semaphore via a throwaway matmul
    # (waits attached to a self-loading matmul land on the MATMUL half of
    # the lowered LDWEIGHTS+MATMUL pair, so the wait must ride an earlier
    # Tensor-queue instruction to also gate the first real weight load).
    t0sz = tile_sizes[0]
    t1sz = tile_sizes[1]
    z0 = nc.alloc_sbuf_tensor("z0raw", [128, t0sz, D], mybir.dt.float8e4)
    z1 = nc.alloc_sbuf_tensor("z1raw", [128, t1sz, D], mybir.dt.float8e4)
    z0sem = nc.alloc_semaphore("z0sem")
    z1sem = nc.alloc_semaphore("z1sem")
    nc.sync.dma_start(
        z0.ap(), z[0 : CHUNK * t0sz, :].rearrange("(p k) d -> p k d", p=128)
    ).then_inc(z0sem, 16)
    nc.scalar.dma_start(
        z1.ap(),
        z[CHUNK * t0sz : CHUNK * (t0sz + t1sz), :].rearrange(
            "(p k) d -> p k d", p=128
        ),
    ).then_inc(z1sem, 16)

    with tile.TileContext(nc) as tc:
        with ExitStack() as ctx:
            psum = ctx.enter_context(tc.tile_pool(name="psum", bufs=1, space="PSUM"))
            opool = ctx.enter_context(tc.tile_pool(name="o", bufs=1))
            accs = [
                psum.tile([128, (hi - lo) * D], mybir.dt.float32, name=f"acc{gi}")
                for gi, (lo, hi) in enumerate(groups)
            ]
            scratch = psum.tile([128, 1], mybir.dt.float32)
            sb_out = opool.tile([128, C * D], mybir.dt.float16)
            # the gate: loads garbage weights ungated (never read), then its
            # MATMUL carries the z0sem wait (attached post-scheduling) and,
            # because the Tensor queue is in-order, blocks every later
            # LDWEIGHTS until tile 0's data has landed
            gate_mm = nc.tensor.matmul(
                scratch[:],
                z0.ap()[:, 0:2, :],
                z0.ap()[:, 0:2, 0:1],
                start=True,
                stop=True,
                perf_mode=mybir.MatmulPerfMode.DoubleRow,
                skip_group_check=True,
            )
            pi = 0  # next matmul in plan
            row0 = 0
            gate2 = None
            for t, tsz in enumerate(tile_sizes):
                if t == 0:
                    tl = z0.ap()
                elif t == 1:
                    tl = z1.ap()
                    # second gate: blocks tile 1's weight loads until its
                    # raw DMA lands (in-order Tensor queue)
                    gate2 = nc.tensor.matmul(
                        scratch[:],
                        z1.ap()[:, 0:2, :],
                        z1.ap()[:, 0:2, 0:1],
                        start=True,
                        stop=True,
                        perf_mode=mybir.MatmulPerfMode.DoubleRow,
                        skip_group_check=True,
                    )
                else:
                    pool = ctx.enter_context(tc.tile_pool(name=f"z{t}", bufs=1))
                    tl = pool.tile([128, tsz, D], mybir.dt.float8e4)
                    src = z[row0 : row0 + CHUNK * tsz, :].rearrange(
                        "(p k) d -> p k d", p=128
                    )
                    # three input channels: the two HWDGE rings alternate
                    # (sync first — it picks up ~0.6us sooner after the
                    # preamble), and gpsimd SWDGE carries one early tile to
                    # relieve the ramp crunch
                    eng = {2: nc.gpsimd}.get(
                        t, nc.sync if t % 2 == 0 else nc.scalar
                    )
                    eng.dma_start(tl[:], src)
                tile_lo = row0 // CHUNK
                row0 += CHUNK * tsz
                while pi < len(plan) and plan[pi][0] + plan[pi][1] <= tile_lo + tsz:
                    pos, sz, j, is_first, is_last = plan[pi]
                    k = pos - tile_lo
                    g = next(gi for gi, (lo_, hi_) in enumerate(groups) if lo_ <= j < hi_)
                    lo = groups[g][0]
                    acc = accs[g]
                    sl = tl[:, k : k + sz, :]
                    nc.tensor.matmul(
                        acc[:, (j - lo) * D : (j - lo + 1) * D],
                        sl,
                        sl,
                        start=is_first,
                        stop=is_last,
                        perf_mode=(
                            mybir.MatmulPerfMode.DoubleRow if sz == 2 else None
                        ),
                        skip_group_check=True,
                    )
                    # drain finished PSUM bank groups so the DVE read never
                    # shares a bank with in-flight PE writes
                    if is_last and j == groups[g][1] - 1:
                        sl_o = slice(lo * D, groups[g][1] * D)
                        nc.vector.tensor_copy(sb_out[:, sl_o], acc[:])
                        # store layout so nothing outlasts class 9's tail:
                        # classes 0..7 merge into one scalar-ring store as
                        # soon as class 7's copy lands (scalar input done by
                        # then, flight finishes ~2us before the end); class
                        # 8's 32KB rides idle gpsimd; only class 9's
                        # 256B/partition store trails the last matmul
                        if g == 1:
                            nc.scalar.dma_start(
                                out[:, 0 : 8 * D], sb_out[:, 0 : 8 * D]
                            )
                        elif g == 2:
                            nc.gpsimd.dma_start(out[:, sl_o], sb_out[:, sl_o])
                        elif g == 3:
                            nc.sync.dma_start(out[:, sl_o], sb_out[:, sl_o])
                    pi += 1
    # attach the gates' waits AFTER the tile scheduler ran (its simulator
    # can't see the external DMAs and would report a deadlock), then reset
    # the manual semaphores so back-to-back NEFF executions start from zero
    gate_mm.wait_op(z0sem, 16, "sem-ge")
    gate2.wait_op(z1sem, 16, "sem-ge")
    nc.gpsimd.sem_clear(z0sem)
    nc.gpsimd.sem_clear(z1sem)
    assert pi == len(plan)
    nc.compile()
    return nc


def _is_one_hot(Pi):
    if not (Pi.sum(axis=1) == 1.0).all():
        return False
    if not (Pi.max(axis=1) == 1.0).all():
        return False
    return np.count_nonzero(Pi) == Pi.shape[0]


def _fast_path_M(Z, Pi):
    """Per-class second moments via the device. Returns M [C, D, D] fp64."""
    import ml_dtypes
    from concourse.bass_utils import run_bass_kernel_spmd

    labels = np.argmax(Pi, axis=1)

    # balance every class across cores: class j's rows are dealt out in
    # near-equal contiguous slices, so per-class per-core counts differ by
    # at most 1 and padding is minimal
    order = np.argsort(labels, kind="stable")
    cls_counts = np.bincount(labels, minlength=C)
    cls_offs = np.concatenate([[0], np.cumsum(cls_counts)])

    counts = np.zeros((N_CORES, C), dtype=np.int64)
    for j in range(C):
        m = cls_counts[j]
        base, rem = divmod(m, N_CORES)
        for c in range(N_CORES):
            counts[c, j] = base + (1 if c < rem else 0)

    seg_sub = [max(1, int(np.ceil(counts[:, j].max() / CHUNK))) for j in range(C)]
    total_sub = sum(seg_sub)
    tile_sizes = _dma_tile_sizes(seg_sub)
    _, plain_pos, dr_pos = _matmul_plan(seg_sub)

    key = tuple(seg_sub)
    if key not in _compiled_cache:
        _compiled_cache[key] = _build_bass_program(seg_sub)
    nc = _compiled_cache[key]

    # ship fp8e4m3: quarters HBM traffic vs fp32 and doubles PE throughput
    # via DoubleRow; the rounding effect on the final losses is ~1.6e-3
    # relative (measured), an order of magnitude inside the gate
    Zb = Z.astype(ml_dtypes.float8_e4m3)
    in_maps = []
    for c in range(N_CORES):
        zbuf = np.zeros((total_sub * CHUNK, D), dtype=ml_dtypes.float8_e4m3)
        for j in range(C):
            lo = cls_offs[j] + counts[:c, j].sum()
            nj = counts[c, j]
            rows = Zb[order[lo : lo + nj]]
            if j in plain_pos:
                take = min(nj, CHUNK)
                p0 = plain_pos[j] * CHUNK
                zbuf[p0 : p0 + take] = rows[:take]
                rows = rows[take:]
            d0 = dr_pos[j] * CHUNK
            zbuf[d0 : d0 + len(rows)] = rows
        # pre-tile each DMA block: [tsz, 128, D] -> [128, tsz*D]
        parts = []
        start = 0
        for tsz in tile_sizes:
            blk = zbuf[start * CHUNK : (start + tsz) * CHUNK]
            parts.append(
                np.ascontiguousarray(
                    blk.reshape(tsz, CHUNK, D).transpose(1, 0, 2)
                ).reshape(-1)
            )
            start += tsz
        zdev = np.concatenate(parts).reshape(total_sub * CHUNK, D)
        in_maps.append({"z": zdev})

    res = run_bass_kernel_spmd(nc, in_maps, list(range(N_CORES)))
    M = np.zeros((C, D, D), dtype=np.float64)
    for c in range(N_CORES):
        o = res.results[c]["m_out"].astype(np.float64)  # [D, C*D]
        M += o.reshape(D, C, D).transpose(1, 0, 2)
    return M


def _dense_path_M(Z, Pi):
    """General dense Pi: host BLAS contraction. Returns (M, gram) fp64."""
    Zf = np.ascontiguousarray(Z, dtype=np.float32)
    A = (Pi[:, :, None].astype(np.float32) * Zf[:, None, :]).reshape(Zf.shape[0], -1)
    M = (A.T @ Zf).reshape(C, D, D).astype(np.float64)
    gram = (Zf.T @ Zf).astype(np.float64)
    return M, gram


def kernel(Z, Pi, Us):
    Z = np.asarray(Z, dtype=np.float32)
    Pi = np.asarray(Pi, dtype=np.float32)
    Us = np.asarray(Us, dtype=np.float32)
    n, d = Z.shape

    if n == N_TOTAL and d == D and Pi.shape == (n, C) and _is_one_hot(Pi):
        M = _fast_path_M(Z, Pi)
        gram = M.sum(axis=0)
    else:
        M, gram = _dense_path_M(Z, Pi)

    nf = float(n)
    df = float(d)

    A = np.eye(d, dtype=np.float64) + (df / (nf * EPS)) * gram
    sign, logabsdet = np.linalg.slogdet(A)
    loss_R = 0.5 * logabsdet

    trPi = Pi.astype(np.float64).sum(axis=0)
    col_norms_sq = (Us.astype(np.float64) ** 2).sum(axis=1)  # [C, d]
    with np.errstate(divide="ignore"):
        per_class = np.log1p((df / (trPi[:, None] * EPS)) * col_norms_sq).sum(axis=1)
    loss_Rc = ((trPi / (2.0 * nf)) * per_class).sum()

    Us64 = Us.astype(np.float64)
    UUt = np.einsum("jdk,jek->jde", Us64, Us64)
    loss_reg = 0.5 * MU * ((M - UUt) ** 2).sum()

    loss_obj = loss_R - loss_Rc - loss_reg
    return (
        np.float32(-loss_obj),
        np.float32(loss_R),
        np.float32(loss_Rc),
        np.float32(loss_reg),
    )

